# revision 10
# baseline (speedup 1.0000x reference)
"""DSSM (vision Mamba block) Trainium2 kernel.

Problem: B=4, H=W=48, L=2304, D_MODEL=96, D_INNER=192, N=16, R=6, K=3.

Sharding (8 cores, no device-to-device comms):
  core c -> batch b=c//2, d_inner half=c%2. Each core runs the full-d
  front-end (in_proj, depthwise conv, x_dbl) for its batch, the K=3
  selective scans for its 96 d_inner channels, and a partial out_proj
  (contraction over its d-half). Host sums the two partials per batch.

v2 design notes:
  - All front-end matmuls run in bf16 or fp32r (1 PE cycle/row).
  - Direction permutes (k=1 transpose, k=2 reversal) are pure access
    patterns on the scan instruction; every other tensor stays in
    natural (row-major) order, so no permute copies and no un-permutes.
  - y accumulates across all 3 directions and 12 groups in 5 persistent
    PSUM banks; one fused epilogue (u*Ds_sum + y)*z -> out_proj.
  - The PSUM->SBUF b-cast copies (dub) run on the otherwise idle GpSimd
    engine; exp(A*delta) runs on ACT; scan/muls on DVE.
"""

import numpy as np
import ml_dtypes

import concourse.bass as bass
import concourse.mybir as mybir
import concourse.tile as tile
from concourse.bass_utils import run_bass_kernel_spmd

# ---------------------------------------------------------------- tile fix
# The walrus here accepts only ONE inline sem-wait per instruction; Tile can
# attach several. Hoist extras onto same-engine NOPs placed just before.
_wsplit_counter = [0]


def _split_multi_waits(nc):
    for fn in nc.m.functions:
        for blk in fn.blocks:
            out = []
            changed = False
            for inst in blk.instructions:
                si = inst.sync_info
                waits = list(si.on_wait) if si is not None and si.on_wait else []
                if len(waits) > 1:
                    changed = True
                    for w in waits[:-1]:
                        _wsplit_counter[0] += 1
                        nop = mybir.InstNoOp(name=f"wsplit-{_wsplit_counter[0]}")
                        nop.engine = inst.engine
                        nop.sync_info = mybir.SyncInfo(on_wait=[w], on_update=[])
                        out.append(nop)
                    inst.sync_info = mybir.SyncInfo(
                        on_wait=[waits[-1]],
                        on_update=list(si.on_update) if si.on_update else [],
                    )
                out.append(inst)
            if changed:
                blk.instructions = out


class TileContextFixed(tile.TileContext):
    def __exit__(self, exc_type, exc_val, exc_tb):
        r = super().__exit__(exc_type, exc_val, exc_tb)
        if exc_type is None:
            _split_multi_waits(self.nc)
        return r


# ---------------------------------------------------------------- constants
B, H, W = 4, 48, 48
DM, DI, N, R, K = 96, 192, 16, 6, 3
L = H * W
DH = 96          # d-half per core
G = DH // 8      # 12 groups of 8 channels
TILES = [(0, 480), (480, 960), (960, 1440), (1440, 1920), (1920, 2304)]

F32 = mybir.dt.float32
F32R = mybir.dt.float32r
BF16 = mybir.dt.bfloat16
MUL = mybir.AluOpType.mult
ADD = mybir.AluOpType.add
AF = mybir.ActivationFunctionType

_COMPILED = {}


def _build_nc():
    nc = bass.Bass()

    # ---- dram I/O (per-core values supplied via in_maps)
    x_nat = nc.dram_tensor("x_nat", [L, DM], F32, kind="ExternalInput")
    wxz_T = nc.dram_tensor("wxz_T", [DM, 288], BF16, kind="ExternalInput")
    conv_diag = nc.dram_tensor("conv_diag", [DH, 18 * DH], BF16, kind="ExternalInput")
    conv_bias = nc.dram_tensor("conv_bias", [DH, 2], F32, kind="ExternalInput")
    xp_T = nc.dram_tensor("xp_T", [DH, K * 2 * 80], BF16, kind="ExternalInput")
    dtw_T = nc.dram_tensor("dtw_T", [R, K * DH], BF16, kind="ExternalInput")
    dt_bias = nc.dram_tensor("dt_bias", [DH, K], F32, kind="ExternalInput")
    wa8 = nc.dram_tensor("wa8", [DH, K * G * 128], BF16, kind="ExternalInput")
    wr = nc.dram_tensor("wr", [128, G * DH], BF16, kind="ExternalInput")
    wbc = nc.dram_tensor("wbc", [80, 128], BF16, kind="ExternalInput")
    wi8 = nc.dram_tensor("wi8", [DH, G * 128], BF16, kind="ExternalInput")
    ident = nc.dram_tensor("ident", [128, 128], F32, kind="ExternalInput")
    ds_sum = nc.dram_tensor("ds_sum", [DH, 2], F32, kind="ExternalInput")
    wout_T = nc.dram_tensor("wout_T", [DH, DM], BF16, kind="ExternalInput")
    out_part = nc.dram_tensor("out_part", [DM, L], F32, kind="ExternalOutput")

    with TileContextFixed(nc) as tc:
        with (
            tc.tile_pool(name="wts", bufs=1) as wts,
            tc.tile_pool(name="big", bufs=1) as big,
            tc.tile_pool(name="stream", bufs=4) as stream,
            tc.tile_pool(name="gpool", bufs=2) as gpool,
        ):
            # ---- load weights
            def wload(dram, shape, dtype):
                t = wts.tile(shape, dtype, tag=dram.name + "_s", name=dram.name + "_s")
                nc.sync.dma_start(t[:, :], dram[:, :])
                return t

            s_wxz = wload(wxz_T, [DM, 288], BF16)
            s_cd = wload(conv_diag, [DH, 18 * DH], BF16)
            s_cb = wload(conv_bias, [DH, 2], F32)
            s_xp = wload(xp_T, [DH, K * 2 * 80], BF16)
            s_dtw = wload(dtw_T, [R, K * DH], BF16)
            s_dtb = wload(dt_bias, [DH, K], F32)
            s_wa8 = wload(wa8, [DH, K * G * 128], BF16)
            s_wr = wload(wr, [128, G * DH], BF16)
            s_wbc = wload(wbc, [80, 128], BF16)
            s_wi8 = wload(wi8, [DH, G * 128], BF16)
            s_id = wload(ident, [128, 128], F32)
            s_ds = wload(ds_sum, [DH, 2], F32)
            s_wout = wload(wout_T, [DH, DM], BF16)

            with (
                tc.tile_pool(name="psF", bufs=2, space="PSUM") as psF,
                tc.tile_pool(name="psT", bufs=2, space="PSUM") as psT,
            ):
                # ---- x load natural, transpose on PE: (L, 96) -> [96, L]
                xT = big.tile([DM, L], BF16, tag="xT")
                for i in range(L // 128):
                    xn = stream.tile([128, DM], F32, tag="xn")
                    nc.sync.dma_start(xn[:, :], x_nat[128 * i : 128 * i + 128, :])
                    pst = psT.tile([DM, 128], F32, tag="psT")
                    nc.tensor.transpose(pst[:, :], xn[:, :], s_id[:, :])
                    nc.scalar.copy(xT[:, 128 * i : 128 * i + 128], pst[:, :])

                # ---- pads for conv (one per half), zeroed borders, bf16
                pads = [big.tile([DH, 50 * 50], BF16, tag=f"pad{h}", name=f"pad{h}")
                        for h in range(2)]
                for p in pads:
                    nc.gpsimd.memset(p[:, :], 0.0)

                # ---- in_proj (fp32r): xc (both halves, into pad layout) + z
                zs = big.tile([DH, L], BF16, tag="zs")
                for it, (t0, t1) in enumerate(TILES):
                    tw = t1 - t0
                    rows = tw // 48
                    for hh in range(2):
                        ps = psF.tile([DH, 480], F32, tag="psF")
                        nc.tensor.matmul(
                            ps[:, :tw],
                            s_wxz[:, 96 * hh : 96 * hh + 96],
                            xT[:, t0:t1],
                        )
                        dst = pads[hh][:, :].rearrange("p (r c) -> p r c", r=50, c=50)[
                            :, 1 + 10 * it : 1 + 10 * it + rows, 1:49
                        ]
                        src = ps[:, :tw].rearrange("p (r c) -> p r c", r=rows, c=48)
                        nc.scalar.copy(dst, src)
                    ps = psF.tile([DH, 480], F32, tag="psF")
                    nc.tensor.matmul(
                        ps[:, :tw],
                        s_wxz[:, 192:288],
                        xT[:, t0:t1],
                    )
                    nc.scalar.activation(zs[:, t0:t1], ps[:, :tw], AF.Silu)

                # ---- depthwise conv 3x3 (bf16) + bias + silu -> u (per half)
                us = [big.tile([DH, L], BF16, tag=f"u{h}", name=f"u{h}") for h in range(2)]
                for hh in range(2):
                    for rb in range(6):
                        ps = psF.tile([DH, 480], F32, tag="psF")
                        j = 0
                        for dy in range(3):
                            for dx in range(3):
                                src = pads[hh][:, :].rearrange(
                                    "p (r c) -> p r c", r=50, c=50
                                )[:, 8 * rb + dy : 8 * rb + dy + 8, dx : dx + 48]
                                nc.tensor.matmul(
                                    ps[:, :384],
                                    s_cd[:, (hh * 9 + j) * DH : (hh * 9 + j + 1) * DH],
                                    src,
                                    start=(j == 0),
                                    stop=(j == 8),
                                )
                                j += 1
                        nc.scalar.activation(
                            us[hh][:, rb * 384 : rb * 384 + 384],
                            ps[:, :384],
                            AF.Silu,
                            bias=s_cb[:, hh : hh + 1],
                        )

                # ---- x_dbl per direction (bf16): [80, L], dt@0 B@32 C@64
                # Direction permutes live in the matmul rhs access patterns:
                # xdbl_1 is stored w-major (transposed), xdbl_2 reversed, so
                # every downstream per-k tensor is already in scan order.
                def u_src(hh, k, t0, t1):
                    if k == 0:
                        return us[hh][:, t0:t1]
                    if k == 1:
                        return us[hh][:, :].rearrange(
                            "p (h w) -> p w h", h=H, w=W
                        )[:, t0 // 48 : t1 // 48, :]
                    return us[hh][:, L - t1 : L - t0][:, ::-1]

                xdbls = [big.tile([80, L], BF16, tag=f"xdbl{k}", name=f"xdbl{k}")
                         for k in range(K)]
                for t0, t1 in TILES:
                    tw = t1 - t0
                    for k in range(K):
                        ps = psF.tile([80, 480], F32, tag="psF2", name="psF2")
                        nc.tensor.matmul(
                            ps[:, :tw], s_xp[:, (2 * k) * 80 : (2 * k + 1) * 80],
                            u_src(0, k, t0, t1), start=True, stop=False,
                        )
                        nc.tensor.matmul(
                            ps[:, :tw], s_xp[:, (2 * k + 1) * 80 : (2 * k + 2) * 80],
                            u_src(1, k, t0, t1), start=False, stop=True,
                        )
                        nc.scalar.copy(xdbls[k][:, t0:t1], ps[:, :tw])

                # ---- delta (softplus) and du per direction (in scan order)
                deltas = [big.tile([DH, L], BF16, tag=f"delta{k}", name=f"delta{k}")
                          for k in range(K)]
                dus = [big.tile([DH, L], BF16, tag=f"du{k}", name=f"du{k}")
                       for k in range(K)]
                for k in range(K):
                    for t0, t1 in TILES:
                        tw = t1 - t0
                        ps = psF.tile([DH, 480], F32, tag="psF")
                        nc.tensor.matmul(
                            ps[:, :tw],
                            s_dtw[:, k * DH : (k + 1) * DH],
                            xdbls[k][0:R, t0:t1],
                        )
                        ev = stream.tile([DH, 480], F32, tag="ev")
                        nc.scalar.activation(
                            ev[:, :tw], ps[:, :tw], AF.Exp,
                            bias=s_dtb[:, k : k + 1],
                        )
                        nc.scalar.activation(
                            deltas[k][:, t0:t1], ev[:, :tw], AF.Ln, bias=1.0
                        )
                    if k == 0:
                        nc.vector.tensor_mul(
                            dus[k][:, :], deltas[k][:, :], us[0][:, :]
                        )
                    elif k == 1:
                        nc.vector.tensor_mul(
                            dus[k][:, :].rearrange("p (a b) -> p a b", a=W, b=H),
                            deltas[k][:, :].rearrange("p (a b) -> p a b", a=W, b=H),
                            us[0][:, :].rearrange("p (h w) -> p w h", h=H, w=W),
                        )
                    else:
                        nc.vector.tensor_mul(
                            dus[k][:, :], deltas[k][:, :], us[0][:, ::-1]
                        )

                # ---- B/C partition-broadcasts (n-minor): [128, L] bf16
                b_bs = [big.tile([128, L], BF16, tag=f"b_b{k}", name=f"b_b{k}")
                        for k in range(K)]
                c_bs = [big.tile([128, L], BF16, tag=f"c_b{k}", name=f"c_b{k}")
                        for k in range(K)]
                for k in range(K):
                    for t0, t1 in TILES:
                        tw = t1 - t0
                        psb = psT.tile([128, 480], F32, tag="psT2", name="psT2")
                        nc.tensor.matmul(psb[:, :tw], s_wbc[32:48, :], xdbls[k][32:48, t0:t1])
                        nc.scalar.copy(b_bs[k][:, t0:t1], psb[:, :tw])
                        psc = psT.tile([128, 480], F32, tag="psT2", name="psT2")
                        nc.tensor.matmul(psc[:, :tw], s_wbc[64:80, :], xdbls[k][64:80, t0:t1])
                        nc.scalar.copy(c_bs[k][:, t0:t1], psc[:, :tw])

            # ================= scan section =================
            with (
                tc.tile_pool(name="psY", bufs=1, space="PSUM") as psY,
                tc.tile_pool(name="psa", bufs=1, space="PSUM") as psa,
                tc.tile_pool(name="psd", bufs=2, space="PSUM") as psd,
            ):
                psy_t = [psY.tile([DH, TILES[c][1] - TILES[c][0]], F32,
                                  tag=f"psy{c}", name=f"psy{c}") for c in range(5)]
                for k in range(K):
                    for g in range(G):
                        gi = k * G + g
                        # work distribution knobs (balance ACT/DVE/Pool)
                        dub_on_act = (gi % 5) != 0   # else: DVE 1x mul from PSUM
                        ch_on_pool = (gi % 18) != 0

                        a_t = gpool.tile([128, L], BF16, tag="a")
                        dub = gpool.tile([128, L], BF16, tag="dub")
                        w_t = gpool.tile([128, L], BF16, tag="w")
                        for t0, t1 in TILES:
                            tw = t1 - t0
                            pa = psa.tile([128, 480], F32, tag="psa")
                            nc.tensor.matmul(
                                pa[:, :tw],
                                s_wa8[:, (k * G + g) * 128 : (k * G + g + 1) * 128],
                                deltas[k][:, t0:t1],
                            )
                            nc.scalar.activation(a_t[:, t0:t1], pa[:, :tw], AF.Exp)
                            pd = psd.tile([128, 480], F32, tag="psd")
                            nc.tensor.matmul(
                                pd[:, :tw],
                                s_wi8[:, g * 128 : (g + 1) * 128],
                                dus[k][:, t0:t1],
                            )
                            if dub_on_act:
                                nc.scalar.copy(dub[:, t0:t1], pd[:, :tw])
                            else:
                                nc.vector.tensor_mul(
                                    w_t[:, t0:t1], pd[:, :tw], b_bs[k][:, t0:t1]
                                )
                        if dub_on_act:
                            nc.vector.tensor_mul(w_t[:, :], dub[:, :], b_bs[k][:, :])
                        h_t = gpool.tile([128, L], BF16, tag="h")
                        nc.vector.tensor_tensor_scan(
                            h_t[:, :], a_t[:, :], w_t[:, :], 0.0, MUL, ADD
                        )
                        ch = gpool.tile([128, L], BF16, tag="ch")
                        if ch_on_pool:
                            nc.gpsimd.tensor_mul(ch[:, :], h_t[:, :], c_bs[k][:, :])
                        else:
                            nc.vector.tensor_mul(ch[:, :], h_t[:, :], c_bs[k][:, :])
                        # un-permute via the psy rhs access pattern
                        for c, (t0, t1) in enumerate(TILES):
                            tw = t1 - t0
                            if k == 0:
                                rhs = ch[:, t0:t1]
                            elif k == 1:
                                rhs = ch[:, :].rearrange(
                                    "p (w h) -> p h w", w=W, h=H
                                )[:, t0 // 48 : t1 // 48, :]
                            else:
                                rhs = ch[:, L - t1 : L - t0][:, ::-1]
                            nc.tensor.matmul(
                                psy_t[c][:, :tw],
                                s_wr[:, g * DH : (g + 1) * DH],
                                rhs,
                                start=(k == 0 and g == 0),
                                stop=(k == K - 1 and g == G - 1),
                            )

                # ---- epilogue: ys = u*Ds_sum + y ; gate *z ; out_proj
                ys = big.tile([DH, L], BF16, tag="ys")
                for c, (t0, t1) in enumerate(TILES):
                    tw = t1 - t0
                    nc.vector.scalar_tensor_tensor(
                        ys[:, t0:t1], us[0][:, t0:t1], s_ds[:, 0:1],
                        psy_t[c][:, :tw], MUL, ADD,
                    )
                yg = big.tile([DH, L], BF16, tag="yg")
                nc.vector.tensor_mul(yg[:, :], ys[:, :], zs[:, :])

                out_sb = big.tile([DM, L], F32, tag="out_sb")
                for t0, t1 in TILES:
                    tw = t1 - t0
                    po = psa.tile([128, 480], F32, tag="psa")
                    nc.tensor.matmul(po[0:DM, :tw], s_wout[:, :], yg[:, t0:t1])
                    nc.scalar.copy(out_sb[:, t0:t1], po[0:DM, :tw])
                    nc.sync.dma_start(out_part[:, t0:t1], out_sb[:, t0:t1])

    return nc


def _prep_in_maps(inputs):
    f32 = lambda a: np.ascontiguousarray(np.asarray(a, np.float32))
    bf16 = lambda a: np.ascontiguousarray(
        np.asarray(a, np.float32).astype(ml_dtypes.bfloat16)
    )
    x = f32(inputs["x"])
    in_proj_w = f32(inputs["in_proj_w"])        # (384, 96)
    conv_w = f32(inputs["conv_w"]).reshape(DI, 9)
    conv_b = f32(inputs["conv_b"])
    x_proj_w = f32(inputs["x_proj_w"])          # (K, 38, 192)
    dt_w = f32(inputs["dt_projs_w"])            # (K, 192, 6)
    dt_b = f32(inputs["dt_projs_b"])            # (K, 192)
    A = -np.exp(f32(inputs["A_logs"])).reshape(K, DI, N)
    Ds = f32(inputs["Ds"]).reshape(K, DI)
    out_w = f32(inputs["out_proj_w"])           # (96, 192)

    wr_np = np.zeros((128, G * DH), np.float32)
    for g in range(G):
        for d8 in range(8):
            wr_np[d8 * 16 : d8 * 16 + 16, g * DH + g * 8 + d8] = 1.0

    in_maps = []
    for c in range(8):
        b, half = c // 2, c % 2
        pd = np.concatenate([np.arange(DI)[96 * half : 96 * half + 96],
                             np.arange(DI)[96 * (1 - half) : 96 * (1 - half) + 96]])
        dh = pd[:DH]

        wxz = np.zeros((DM, 288), np.float32)
        wxz[:, 0:96] = in_proj_w[pd[:96]].T
        wxz[:, 96:192] = in_proj_w[pd[96:]].T
        wxz[:, 192:288] = in_proj_w[DI + dh].T

        cd = np.zeros((DH, 18 * DH), np.float32)
        for hh in range(2):
            ch_idx = pd[hh * 96 : hh * 96 + 96]
            for j in range(9):
                blk = np.zeros((DH, DH), np.float32)
                np.fill_diagonal(blk, conv_w[ch_idx, j])
                cd[:, (hh * 9 + j) * DH : (hh * 9 + j + 1) * DH] = blk
        cb = np.stack([conv_b[pd[:96]], conv_b[pd[96:]]], axis=1)

        xp = np.zeros((DH, K * 2 * 80), np.float32)
        for k in range(K):
            for hh in range(2):
                blk = np.zeros((DH, 80), np.float32)
                ch_idx = pd[hh * 96 : hh * 96 + 96]
                blk[:, 0:6] = x_proj_w[k][0:6, ch_idx].T
                blk[:, 32:48] = x_proj_w[k][6:22, ch_idx].T
                blk[:, 64:80] = x_proj_w[k][22:38, ch_idx].T
                xp[:, (2 * k + hh) * 80 : (2 * k + hh + 1) * 80] = blk

        dtw = np.zeros((R, K * DH), np.float32)
        for k in range(K):
            dtw[:, k * DH : (k + 1) * DH] = dt_w[k][dh].T
        dtb = np.stack([dt_b[k][dh] for k in range(K)], axis=1)

        wa = np.zeros((DH, K * G * 128), np.float32)
        for k in range(K):
            for g in range(G):
                blk = np.zeros((DH, 128), np.float32)
                for d8 in range(8):
                    blk[g * 8 + d8, d8 * 16 : d8 * 16 + 16] = A[k, dh[g * 8 + d8]]
                wa[:, (k * G + g) * 128 : (k * G + g + 1) * 128] = blk

        ds_np = np.zeros((DH, 2), np.float32)
        ds_np[:, 0] = sum(Ds[k][dh] for k in range(K))
        ds_np[:, 1] = ds_np[:, 0]

        wi8_np = np.zeros((DH, G * 128), np.float32)
        for g in range(G):
            for d8 in range(8):
                wi8_np[g * 8 + d8, g * 128 + d8 * 16 : g * 128 + d8 * 16 + 16] = 1.0

        wbc_np = np.zeros((80, 128), np.float32)
        for n in range(16):
            wbc_np[32 + n, n::16] = 1.0
            wbc_np[64 + n, n::16] = 1.0

        in_maps.append(
            dict(
                x_nat=x[b].reshape(L, DM),
                wxz_T=wxz.astype(ml_dtypes.bfloat16),
                conv_diag=cd.astype(ml_dtypes.bfloat16),
                conv_bias=np.ascontiguousarray(cb),
                xp_T=xp.astype(ml_dtypes.bfloat16),
                dtw_T=dtw.astype(ml_dtypes.bfloat16),
                dt_bias=np.ascontiguousarray(dtb),
                wa8=wa.astype(ml_dtypes.bfloat16),
                wr=wr_np.astype(ml_dtypes.bfloat16),
                wbc=wbc_np.astype(ml_dtypes.bfloat16),
                wi8=wi8_np.astype(ml_dtypes.bfloat16),
                ident=np.eye(128, dtype=np.float32),
                ds_sum=ds_np,
                wout_T=np.ascontiguousarray(out_w[:, dh].T).astype(ml_dtypes.bfloat16),
            )
        )
    return in_maps


def kernel(**inputs):
    if "nc" not in _COMPILED:
        _COMPILED["nc"] = _build_nc()
    nc = _COMPILED["nc"]
    in_maps = _prep_in_maps(inputs)
    res = run_bass_kernel_spmd(nc, in_maps, core_ids=list(range(8)))
    out = np.zeros((B, H, W, DM), np.float32)
    for b in range(B):
        p = res.results[2 * b]["out_part"] + res.results[2 * b + 1]["out_part"]
        out[b] = p.T.reshape(H, W, DM)
    return out


# revision 12
# speedup vs baseline: 1.0801x; 1.0801x over previous
"""DSSM (vision Mamba block) Trainium2 kernel.

Problem: B=4, H=W=48, L=2304, D_MODEL=96, D_INNER=192, N=16, R=6, K=3.

Sharding (8 cores, no device-to-device comms):
  core c -> batch b=c//2, d_inner half=c%2. Each core runs the full-d
  front-end (in_proj, depthwise conv, x_dbl) for its batch, the K=3
  selective scans for its 96 d_inner channels, and a partial out_proj
  (contraction over its d-half). Host sums the two partials per batch.

v2 design notes:
  - All front-end matmuls run in bf16 or fp32r (1 PE cycle/row).
  - Direction permutes (k=1 transpose, k=2 reversal) are pure access
    patterns on the scan instruction; every other tensor stays in
    natural (row-major) order, so no permute copies and no un-permutes.
  - y accumulates across all 3 directions and 12 groups in 5 persistent
    PSUM banks; one fused epilogue (u*Ds_sum + y)*z -> out_proj.
  - The PSUM->SBUF b-cast copies (dub) run on the otherwise idle GpSimd
    engine; exp(A*delta) runs on ACT; scan/muls on DVE.
"""

import numpy as np
import ml_dtypes

import concourse.bass as bass
import concourse.mybir as mybir
import concourse.tile as tile
from concourse.bass_utils import run_bass_kernel_spmd

# ---------------------------------------------------------------- tile fix
# The walrus here accepts only ONE inline sem-wait per instruction; Tile can
# attach several. Hoist extras onto same-engine NOPs placed just before.
_wsplit_counter = [0]


def _split_multi_waits(nc):
    for fn in nc.m.functions:
        for blk in fn.blocks:
            out = []
            changed = False
            for inst in blk.instructions:
                si = inst.sync_info
                waits = list(si.on_wait) if si is not None and si.on_wait else []
                if len(waits) > 1:
                    changed = True
                    for w in waits[:-1]:
                        _wsplit_counter[0] += 1
                        nop = mybir.InstNoOp(name=f"wsplit-{_wsplit_counter[0]}")
                        nop.engine = inst.engine
                        nop.sync_info = mybir.SyncInfo(on_wait=[w], on_update=[])
                        out.append(nop)
                    inst.sync_info = mybir.SyncInfo(
                        on_wait=[waits[-1]],
                        on_update=list(si.on_update) if si.on_update else [],
                    )
                out.append(inst)
            if changed:
                blk.instructions = out


class TileContextFixed(tile.TileContext):
    def __exit__(self, exc_type, exc_val, exc_tb):
        r = super().__exit__(exc_type, exc_val, exc_tb)
        if exc_type is None:
            _split_multi_waits(self.nc)
        return r


# ---------------------------------------------------------------- constants
B, H, W = 4, 48, 48
DM, DI, N, R, K = 96, 192, 16, 6, 3
L = H * W
DH = 96          # d-half per core
G = DH // 8      # 12 groups of 8 channels
TILES = [(0, 480), (480, 960), (960, 1440), (1440, 1920), (1920, 2304)]

F32 = mybir.dt.float32
F32R = mybir.dt.float32r
BF16 = mybir.dt.bfloat16
MUL = mybir.AluOpType.mult
ADD = mybir.AluOpType.add
AF = mybir.ActivationFunctionType

_COMPILED = {}


def _build_nc():
    nc = bass.Bass()

    # ---- dram I/O (per-core values supplied via in_maps)
    x_nat = nc.dram_tensor("x_nat", [L, DM], F32, kind="ExternalInput")
    wxz_T = nc.dram_tensor("wxz_T", [DM, 288], BF16, kind="ExternalInput")
    conv_diag = nc.dram_tensor("conv_diag", [DH, 18 * DH], BF16, kind="ExternalInput")
    conv_bias = nc.dram_tensor("conv_bias", [DH, 2], F32, kind="ExternalInput")
    xp_T = nc.dram_tensor("xp_T", [DH, K * 2 * 80], BF16, kind="ExternalInput")
    dtw_T = nc.dram_tensor("dtw_T", [R, K * DH], BF16, kind="ExternalInput")
    dt_bias = nc.dram_tensor("dt_bias", [DH, K], F32, kind="ExternalInput")
    wa8 = nc.dram_tensor("wa8", [DH, K * G * 128], BF16, kind="ExternalInput")
    wr = nc.dram_tensor("wr", [128, G * DH], BF16, kind="ExternalInput")
    wbc = nc.dram_tensor("wbc", [80, 128], BF16, kind="ExternalInput")
    wi8 = nc.dram_tensor("wi8", [DH, G * 128], BF16, kind="ExternalInput")
    ident = nc.dram_tensor("ident", [128, 128], F32, kind="ExternalInput")
    ds_sum = nc.dram_tensor("ds_sum", [DH, 2], F32, kind="ExternalInput")
    wout_T = nc.dram_tensor("wout_T", [DH, DM], BF16, kind="ExternalInput")
    out_part = nc.dram_tensor("out_part", [DM, L], F32, kind="ExternalOutput")

    with TileContextFixed(nc) as tc:
        with (
            tc.tile_pool(name="wts", bufs=1) as wts,
            tc.tile_pool(name="big", bufs=1) as big,
            tc.tile_pool(name="stream", bufs=4) as stream,
            tc.tile_pool(name="gpool", bufs=3) as gpool,
        ):
            # ---- load weights
            def wload(dram, shape, dtype):
                t = wts.tile(shape, dtype, tag=dram.name + "_s", name=dram.name + "_s")
                nc.sync.dma_start(t[:, :], dram[:, :])
                return t

            s_wxz = wload(wxz_T, [DM, 288], BF16)
            s_cd = wload(conv_diag, [DH, 18 * DH], BF16)
            s_cb = wload(conv_bias, [DH, 2], F32)
            s_xp = wload(xp_T, [DH, K * 2 * 80], BF16)
            s_dtw = wload(dtw_T, [R, K * DH], BF16)
            s_dtb = wload(dt_bias, [DH, K], F32)
            s_wa8 = wload(wa8, [DH, K * G * 128], BF16)
            s_wr = wload(wr, [128, G * DH], BF16)
            s_wbc = wload(wbc, [80, 128], BF16)
            s_wi8 = wload(wi8, [DH, G * 128], BF16)
            s_id = wload(ident, [128, 128], F32)
            s_ds = wload(ds_sum, [DH, 2], F32)
            s_wout = wload(wout_T, [DH, DM], BF16)

            with (
                tc.tile_pool(name="psF", bufs=2, space="PSUM") as psF,
                tc.tile_pool(name="psT", bufs=2, space="PSUM") as psT,
            ):
                # ---- x load natural, transpose on PE: (L, 96) -> [96, L]
                xT = big.tile([DM, L], BF16, tag="xT")
                for i in range(L // 128):
                    xn = stream.tile([128, DM], F32, tag="xn")
                    nc.sync.dma_start(xn[:, :], x_nat[128 * i : 128 * i + 128, :])
                    pst = psT.tile([DM, 128], F32, tag="psT")
                    nc.tensor.transpose(pst[:, :], xn[:, :], s_id[:, :])
                    nc.scalar.copy(xT[:, 128 * i : 128 * i + 128], pst[:, :])

                # ---- pads for conv (one per half), zeroed borders, bf16
                pads = [big.tile([DH, 50 * 50], BF16, tag=f"pad{h}", name=f"pad{h}")
                        for h in range(2)]
                for p in pads:
                    nc.gpsimd.memset(p[:, :], 0.0)

                # ---- in_proj (fp32r): xc (both halves, into pad layout) + z
                zs = big.tile([DH, L], BF16, tag="zs")
                for it, (t0, t1) in enumerate(TILES):
                    tw = t1 - t0
                    rows = tw // 48
                    for hh in range(2):
                        ps = psF.tile([DH, 480], F32, tag="psF")
                        nc.tensor.matmul(
                            ps[:, :tw],
                            s_wxz[:, 96 * hh : 96 * hh + 96],
                            xT[:, t0:t1],
                        )
                        dst = pads[hh][:, :].rearrange("p (r c) -> p r c", r=50, c=50)[
                            :, 1 + 10 * it : 1 + 10 * it + rows, 1:49
                        ]
                        src = ps[:, :tw].rearrange("p (r c) -> p r c", r=rows, c=48)
                        nc.scalar.copy(dst, src)
                    ps = psF.tile([DH, 480], F32, tag="psF")
                    nc.tensor.matmul(
                        ps[:, :tw],
                        s_wxz[:, 192:288],
                        xT[:, t0:t1],
                    )
                    nc.scalar.activation(zs[:, t0:t1], ps[:, :tw], AF.Silu)

                # ---- depthwise conv 3x3 (bf16) + bias + silu -> u (per half)
                us = [big.tile([DH, L], BF16, tag=f"u{h}", name=f"u{h}") for h in range(2)]
                for hh in range(2):
                    for rb in range(6):
                        ps = psF.tile([DH, 480], F32, tag="psF")
                        j = 0
                        for dy in range(3):
                            for dx in range(3):
                                src = pads[hh][:, :].rearrange(
                                    "p (r c) -> p r c", r=50, c=50
                                )[:, 8 * rb + dy : 8 * rb + dy + 8, dx : dx + 48]
                                nc.tensor.matmul(
                                    ps[:, :384],
                                    s_cd[:, (hh * 9 + j) * DH : (hh * 9 + j + 1) * DH],
                                    src,
                                    start=(j == 0),
                                    stop=(j == 8),
                                )
                                j += 1
                        nc.scalar.activation(
                            us[hh][:, rb * 384 : rb * 384 + 384],
                            ps[:, :384],
                            AF.Silu,
                            bias=s_cb[:, hh : hh + 1],
                        )

                # ---- x_dbl per direction (bf16): [80, L], dt@0 B@32 C@64
                # Direction permutes live in the matmul rhs access patterns:
                # xdbl_1 is stored w-major (transposed), xdbl_2 reversed, so
                # every downstream per-k tensor is already in scan order.
                def u_src(hh, k, t0, t1):
                    if k == 0:
                        return us[hh][:, t0:t1]
                    if k == 1:
                        return us[hh][:, :].rearrange(
                            "p (h w) -> p w h", h=H, w=W
                        )[:, t0 // 48 : t1 // 48, :]
                    return us[hh][:, L - t1 : L - t0][:, ::-1]

                xdbls = [big.tile([80, L], BF16, tag=f"xdbl{k}", name=f"xdbl{k}")
                         for k in range(K)]
                for t0, t1 in TILES:
                    tw = t1 - t0
                    for k in range(K):
                        ps = psF.tile([80, 480], F32, tag="psF2", name="psF2")
                        nc.tensor.matmul(
                            ps[:, :tw], s_xp[:, (2 * k) * 80 : (2 * k + 1) * 80],
                            u_src(0, k, t0, t1), start=True, stop=False,
                        )
                        nc.tensor.matmul(
                            ps[:, :tw], s_xp[:, (2 * k + 1) * 80 : (2 * k + 2) * 80],
                            u_src(1, k, t0, t1), start=False, stop=True,
                        )
                        nc.scalar.copy(xdbls[k][:, t0:t1], ps[:, :tw])

                # ---- delta (softplus) and du per direction (in scan order)
                deltas = [big.tile([DH, L], BF16, tag=f"delta{k}", name=f"delta{k}")
                          for k in range(K)]
                dus = [big.tile([DH, L], BF16, tag=f"du{k}", name=f"du{k}")
                       for k in range(K)]
                for k in range(K):
                    for t0, t1 in TILES:
                        tw = t1 - t0
                        ps = psF.tile([DH, 480], F32, tag="psF")
                        nc.tensor.matmul(
                            ps[:, :tw],
                            s_dtw[:, k * DH : (k + 1) * DH],
                            xdbls[k][0:R, t0:t1],
                        )
                        ev = stream.tile([DH, 480], F32, tag="ev")
                        nc.scalar.activation(
                            ev[:, :tw], ps[:, :tw], AF.Exp,
                            bias=s_dtb[:, k : k + 1],
                        )
                        nc.scalar.activation(
                            deltas[k][:, t0:t1], ev[:, :tw], AF.Ln, bias=1.0
                        )
                    if k == 0:
                        nc.vector.tensor_mul(
                            dus[k][:, :], deltas[k][:, :], us[0][:, :]
                        )
                    elif k == 1:
                        nc.vector.tensor_mul(
                            dus[k][:, :].rearrange("p (a b) -> p a b", a=W, b=H),
                            deltas[k][:, :].rearrange("p (a b) -> p a b", a=W, b=H),
                            us[0][:, :].rearrange("p (h w) -> p w h", h=H, w=W),
                        )
                    else:
                        nc.vector.tensor_mul(
                            dus[k][:, :], deltas[k][:, :], us[0][:, ::-1]
                        )

                # ---- B/C partition-broadcasts (n-minor): [128, L] bf16
                b_bs = [big.tile([128, L], BF16, tag=f"b_b{k}", name=f"b_b{k}")
                        for k in range(K)]
                c_bs = [big.tile([128, L], BF16, tag=f"c_b{k}", name=f"c_b{k}")
                        for k in range(K)]
                for k in range(K):
                    for t0, t1 in TILES:
                        tw = t1 - t0
                        psb = psT.tile([128, 480], F32, tag="psT2", name="psT2")
                        nc.tensor.matmul(psb[:, :tw], s_wbc[32:48, :], xdbls[k][32:48, t0:t1])
                        nc.scalar.copy(b_bs[k][:, t0:t1], psb[:, :tw])
                        psc = psT.tile([128, 480], F32, tag="psT2", name="psT2")
                        nc.tensor.matmul(psc[:, :tw], s_wbc[64:80, :], xdbls[k][64:80, t0:t1])
                        nc.scalar.copy(c_bs[k][:, t0:t1], psc[:, :tw])

            # ================= scan section =================
            with (
                tc.tile_pool(name="psY", bufs=1, space="PSUM") as psY,
                tc.tile_pool(name="psa", bufs=2, space="PSUM") as psa,
                tc.tile_pool(name="psd", bufs=1, space="PSUM") as psd,
            ):
                psy_t = [psY.tile([DH, TILES[c][1] - TILES[c][0]], F32,
                                  tag=f"psy{c}", name=f"psy{c}") for c in range(5)]
                for k in range(K):
                    for g in range(G):
                        gi = k * G + g
                        # work distribution knobs (balance ACT/DVE/Pool)
                        dub_on_act = True   # else: DVE 1x mul from PSUM
                        ch_on_pool = True

                        a_t = gpool.tile([128, L], BF16, tag="a")
                        dub = gpool.tile([128, L], BF16, tag="dub")
                        w_t = gpool.tile([128, L], BF16, tag="w")
                        for t0, t1 in TILES:
                            tw = t1 - t0
                            pa = psa.tile([128, 480], F32, tag="psa")
                            nc.tensor.matmul(
                                pa[:, :tw],
                                s_wa8[:, (k * G + g) * 128 : (k * G + g + 1) * 128],
                                deltas[k][:, t0:t1],
                            )
                            nc.scalar.activation(a_t[:, t0:t1], pa[:, :tw], AF.Exp)
                            pd = psd.tile([128, 480], F32, tag="psd")
                            nc.tensor.matmul(
                                pd[:, :tw],
                                s_wi8[:, g * 128 : (g + 1) * 128],
                                dus[k][:, t0:t1],
                            )
                            if dub_on_act:
                                nc.scalar.copy(dub[:, t0:t1], pd[:, :tw])
                            else:
                                nc.vector.tensor_mul(
                                    w_t[:, t0:t1], pd[:, :tw], b_bs[k][:, t0:t1]
                                )
                        if dub_on_act:
                            nc.vector.tensor_mul(w_t[:, :], dub[:, :], b_bs[k][:, :])
                        h_t = gpool.tile([128, L], BF16, tag="h")
                        nc.vector.tensor_tensor_scan(
                            h_t[:, :], a_t[:, :], w_t[:, :], 0.0, MUL, ADD
                        )
                        ch = gpool.tile([128, L], BF16, tag="ch")
                        if ch_on_pool:
                            nc.gpsimd.tensor_mul(ch[:, :], h_t[:, :], c_bs[k][:, :])
                        else:
                            nc.vector.tensor_mul(ch[:, :], h_t[:, :], c_bs[k][:, :])
                        # un-permute via the psy rhs access pattern
                        for c, (t0, t1) in enumerate(TILES):
                            tw = t1 - t0
                            if k == 0:
                                rhs = ch[:, t0:t1]
                            elif k == 1:
                                rhs = ch[:, :].rearrange(
                                    "p (w h) -> p h w", w=W, h=H
                                )[:, t0 // 48 : t1 // 48, :]
                            else:
                                rhs = ch[:, L - t1 : L - t0][:, ::-1]
                            nc.tensor.matmul(
                                psy_t[c][:, :tw],
                                s_wr[:, g * DH : (g + 1) * DH],
                                rhs,
                                start=(k == 0 and g == 0),
                                stop=(k == K - 1 and g == G - 1),
                            )

                # ---- epilogue: ys = u*Ds_sum + y ; gate *z ; out_proj
                ys = big.tile([DH, L], BF16, tag="pad0", name="ys")
                for c, (t0, t1) in enumerate(TILES):
                    tw = t1 - t0
                    nc.vector.scalar_tensor_tensor(
                        ys[:, t0:t1], us[0][:, t0:t1], s_ds[:, 0:1],
                        psy_t[c][:, :tw], MUL, ADD,
                    )
                yg = big.tile([DH, L], BF16, tag="pad1", name="yg")
                nc.vector.tensor_mul(yg[:, :], ys[:, :], zs[:, :])

                for t0, t1 in TILES:
                    tw = t1 - t0
                    po = psa.tile([128, 480], F32, tag="psa")
                    nc.tensor.matmul(po[0:DM, :tw], s_wout[:, :], yg[:, t0:t1])
                    ot = stream.tile([DM, 480], F32, tag="ot")
                    nc.scalar.copy(ot[:, :tw], po[0:DM, :tw])
                    nc.sync.dma_start(out_part[:, t0:t1], ot[:, :tw])

    return nc


def _prep_in_maps(inputs):
    f32 = lambda a: np.ascontiguousarray(np.asarray(a, np.float32))
    bf16 = lambda a: np.ascontiguousarray(
        np.asarray(a, np.float32).astype(ml_dtypes.bfloat16)
    )
    x = f32(inputs["x"])
    in_proj_w = f32(inputs["in_proj_w"])        # (384, 96)
    conv_w = f32(inputs["conv_w"]).reshape(DI, 9)
    conv_b = f32(inputs["conv_b"])
    x_proj_w = f32(inputs["x_proj_w"])          # (K, 38, 192)
    dt_w = f32(inputs["dt_projs_w"])            # (K, 192, 6)
    dt_b = f32(inputs["dt_projs_b"])            # (K, 192)
    A = -np.exp(f32(inputs["A_logs"])).reshape(K, DI, N)
    Ds = f32(inputs["Ds"]).reshape(K, DI)
    out_w = f32(inputs["out_proj_w"])           # (96, 192)

    wr_np = np.zeros((128, G * DH), np.float32)
    for g in range(G):
        for d8 in range(8):
            wr_np[d8 * 16 : d8 * 16 + 16, g * DH + g * 8 + d8] = 1.0

    in_maps = []
    for c in range(8):
        b, half = c // 2, c % 2
        pd = np.concatenate([np.arange(DI)[96 * half : 96 * half + 96],
                             np.arange(DI)[96 * (1 - half) : 96 * (1 - half) + 96]])
        dh = pd[:DH]

        wxz = np.zeros((DM, 288), np.float32)
        wxz[:, 0:96] = in_proj_w[pd[:96]].T
        wxz[:, 96:192] = in_proj_w[pd[96:]].T
        wxz[:, 192:288] = in_proj_w[DI + dh].T

        cd = np.zeros((DH, 18 * DH), np.float32)
        for hh in range(2):
            ch_idx = pd[hh * 96 : hh * 96 + 96]
            for j in range(9):
                blk = np.zeros((DH, DH), np.float32)
                np.fill_diagonal(blk, conv_w[ch_idx, j])
                cd[:, (hh * 9 + j) * DH : (hh * 9 + j + 1) * DH] = blk
        cb = np.stack([conv_b[pd[:96]], conv_b[pd[96:]]], axis=1)

        xp = np.zeros((DH, K * 2 * 80), np.float32)
        for k in range(K):
            for hh in range(2):
                blk = np.zeros((DH, 80), np.float32)
                ch_idx = pd[hh * 96 : hh * 96 + 96]
                blk[:, 0:6] = x_proj_w[k][0:6, ch_idx].T
                blk[:, 32:48] = x_proj_w[k][6:22, ch_idx].T
                blk[:, 64:80] = x_proj_w[k][22:38, ch_idx].T
                xp[:, (2 * k + hh) * 80 : (2 * k + hh + 1) * 80] = blk

        dtw = np.zeros((R, K * DH), np.float32)
        for k in range(K):
            dtw[:, k * DH : (k + 1) * DH] = dt_w[k][dh].T
        dtb = np.stack([dt_b[k][dh] for k in range(K)], axis=1)

        wa = np.zeros((DH, K * G * 128), np.float32)
        for k in range(K):
            for g in range(G):
                blk = np.zeros((DH, 128), np.float32)
                for d8 in range(8):
                    blk[g * 8 + d8, d8 * 16 : d8 * 16 + 16] = A[k, dh[g * 8 + d8]]
                wa[:, (k * G + g) * 128 : (k * G + g + 1) * 128] = blk

        ds_np = np.zeros((DH, 2), np.float32)
        ds_np[:, 0] = sum(Ds[k][dh] for k in range(K))
        ds_np[:, 1] = ds_np[:, 0]

        wi8_np = np.zeros((DH, G * 128), np.float32)
        for g in range(G):
            for d8 in range(8):
                wi8_np[g * 8 + d8, g * 128 + d8 * 16 : g * 128 + d8 * 16 + 16] = 1.0

        wbc_np = np.zeros((80, 128), np.float32)
        for n in range(16):
            wbc_np[32 + n, n::16] = 1.0
            wbc_np[64 + n, n::16] = 1.0

        in_maps.append(
            dict(
                x_nat=x[b].reshape(L, DM),
                wxz_T=wxz.astype(ml_dtypes.bfloat16),
                conv_diag=cd.astype(ml_dtypes.bfloat16),
                conv_bias=np.ascontiguousarray(cb),
                xp_T=xp.astype(ml_dtypes.bfloat16),
                dtw_T=dtw.astype(ml_dtypes.bfloat16),
                dt_bias=np.ascontiguousarray(dtb),
                wa8=wa.astype(ml_dtypes.bfloat16),
                wr=wr_np.astype(ml_dtypes.bfloat16),
                wbc=wbc_np.astype(ml_dtypes.bfloat16),
                wi8=wi8_np.astype(ml_dtypes.bfloat16),
                ident=np.eye(128, dtype=np.float32),
                ds_sum=ds_np,
                wout_T=np.ascontiguousarray(out_w[:, dh].T).astype(ml_dtypes.bfloat16),
            )
        )
    return in_maps


def kernel(**inputs):
    if "nc" not in _COMPILED:
        _COMPILED["nc"] = _build_nc()
    nc = _COMPILED["nc"]
    in_maps = _prep_in_maps(inputs)
    res = run_bass_kernel_spmd(nc, in_maps, core_ids=list(range(8)))
    out = np.zeros((B, H, W, DM), np.float32)
    for b in range(B):
        p = res.results[2 * b]["out_part"] + res.results[2 * b + 1]["out_part"]
        out[b] = p.T.reshape(H, W, DM)
    return out
